# revision 16
# baseline (speedup 1.0000x reference)
"""CTC total-loss kernel for Trainium2 (8 NeuronCores) — wavefront v2.

Wavefront alpha recursion with a dead "constants" partition group:

 * Halves t<224 (NB=7 rows x TB=32) and t>=224 (NB=6 rows x TB=48).
   Block-row k lives in partitions 4(k+1)..4(k+1)+3; partitions 0..3 are a
   never-scanned constants group.
 * One stream_shuffle per diagonal moves seam state down 4 partitions into
   gap cells; the wrap at partition group 0 reads the constants group, so
   row 0's virtual t=-1 seed (half 0) / renormed boundary (half 1) arrives
   with NO extra per-diagonal op (one strided prefill per half instead).
 * Per diagonal: shuffle + stt (drive) + scan.  141 diagonals total.
 * Renorm at t=224 (boundary max -> e^TB_LOG), compensated on host.
"""

import numpy as np

import concourse.bass as bass
import concourse.bacc as bacc
import concourse.tile as tile
from concourse import mybir

F32 = mybir.dt.float32
BF16 = mybir.dt.bfloat16

T, B, V, LMAX = 512, 32, 4096, 32
NCORES = 8
BC = B // NCORES            # 4 examples per core
S = 2 * LMAX + 1            # 65 lattice states
NT = (T * BC) // 128        # 16 stream tiles of (128, V)
C_TILT = -1.20
TB_LOG = 58.0

# two asymmetric halves; each: NB rows of TB, rows at partitions 4..4+4*NB-1
HALVES = (
    dict(t0=0, nb=7, tb=32),
    dict(t0=224, nb=6, tb=48),
)
for hs in HALVES:
    hs["th"] = hs["nb"] * hs["tb"]
    hs["cww"] = hs["tb"] + 1
    hs["nslot"] = S + hs["nb"] - 1
    hs["xw"] = (3 + hs["nslot"]) * hs["cww"]   # 3 pad slots
PADS = 3
TH0 = HALVES[0]["th"]       # renorm boundary time

_CACHE = {}


def _build_nc():
    nc = bacc.Bacc(None)
    acts_d = nc.dram_tensor("acts", [T, BC, V], F32, kind="ExternalInput")
    gsk_d = [nc.dram_tensor(f"gsk{h}", [32, hs["nslot"] * hs["tb"]], F32,
                            kind="ExternalInput")
             for h, hs in enumerate(HALVES)]
    skm_d = [nc.dram_tensor(f"skm{h}", [32, hs["nslot"]], F32,
                            kind="ExternalInput")
             for h, hs in enumerate(HALVES)]
    xsk_d = [nc.dram_tensor(f"xsk{h}", [32, hs["nslot"] * hs["cww"]], F32,
                            kind="ExternalOutput")
             for h, hs in enumerate(HALVES)]
    seed_d = nc.dram_tensor("seed", [BC, (HALVES[0]["nslot"] + 2)
                            * HALVES[0]["cww"]], F32, kind="ExternalInput")
    rfac_d = nc.dram_tensor("rfac", [BC, 1], F32, kind="ExternalOutput")
    sums_d = nc.dram_tensor("sums", [128, NT], F32, kind="ExternalOutput")

    acts_rows = acts_d[:].rearrange("t b v -> (t b) v")     # (2048, 4096)

    ADD = mybir.AluOpType.add
    MUL = mybir.AluOpType.mult
    mask_dn4 = [(i - 4) % 32 for i in range(32)]
    mask_up24 = [(i + 24) % 32 for i in range(32)]

    with tile.TileContext(nc) as tc:
        with (
            tc.tile_pool(name="small", bufs=1) as small,
            tc.tile_pool(name="big", bufs=1) as big,
            tc.tile_pool(name="stream", bufs=3) as stream,
            tc.tile_pool(name="stream2", bufs=2) as stream2,
            tc.tile_pool(name="psum", bufs=1, space="PSUM") as psump,
        ):
            # ---------------- persistent tiles ----------------
            E = [big.tile([32, hs["nslot"] * hs["tb"]], BF16, tag=f"E{h}",
                          name=f"E{h}") for h, hs in enumerate(HALVES)]
            G = [big.tile([32, hs["nslot"] * hs["tb"]], F32, tag=f"G{h}",
                          name=f"G{h}") for h, hs in enumerate(HALVES)]
            X = [big.tile([32, hs["xw"]], F32, tag=f"X{h}",
                          name=f"X{h}") for h, hs in enumerate(HALVES)]
            SK = [small.tile([32, hs["nslot"]], F32, tag=f"SK{h}",
                           name=f"SK{h}") for h, hs in enumerate(HALVES)]
            u_t = big.tile([32, max(hs["tb"] for hs in HALVES)], F32, tag="u")

            negc = small.tile([32, 1], F32, tag="negc")
            zbias = small.tile([128, 1], F32, tag="zbias")
            bscw = small.tile([BC, (HALVES[1]["nslot"] + 2)
                               * HALVES[1]["cww"]], F32, tag="bscw")
            bndt = small.tile([32, S], F32, tag="bndt")
            m_t = small.tile([BC, 1], F32, tag="m")
            r0_t = small.tile([BC, 1], F32, tag="r0")
            r_t = small.tile([BC, 1], F32, tag="r")
            sums = small.tile([128, NT], F32, tag="sums")

            warm = small.tile([BC, 4], F32, tag="warm")
            nc.gpsimd.dma_start(out=warm[:], in_=seed_d[:, 0:4])
            # X0 memset first: it gates the half-0 constants prefill
            nc.vector.memset(X[0][:, :], 0.0)
            nc.vector.memset(X[1][:, :], 0.0)
            nc.vector.memset(negc[:], -C_TILT)
            nc.vector.memset(zbias[:], 0.0)
            nc.vector.memset(bscw[:], 0.0)
            # prep loads + half-0 constants prefill ride the sync queue
            # ahead of the stream loads (seed is host-filled: no DVE dep
            # beyond the X0 memset)
            for h in range(2):
                nc.sync.dma_start(out=G[h][:], in_=gsk_d[h][:])
                nc.sync.dma_start(out=SK[h][:], in_=skm_d[h][:])
            nc.sync.dma_start(
                out=X[0][28:32,
                         HALVES[0]["tb"]:HALVES[0]["tb"]
                         + (HALVES[0]["nslot"] + 2) * HALVES[0]["cww"]],
                in_=seed_d[:])
            e0cut = 8 * HALVES[0]["tb"]
            nc.scalar.activation(out=E[0][:, 0:e0cut], in_=G[0][:, 0:e0cut],
                                 func=mybir.ActivationFunctionType.Exp,
                                 bias=negc[:], scale=1.0)
            nc.scalar.activation(out=E[0][:, e0cut:], in_=G[0][:, e0cut:],
                                 func=mybir.ActivationFunctionType.Exp,
                                 bias=negc[:], scale=1.0)
            nc.scalar.activation(out=E[1][:], in_=G[1][:],
                                 func=mybir.ActivationFunctionType.Exp,
                                 bias=negc[:], scale=1.0)

            # ---------------- lse stream (DMA-roofline bound) ----------
            # tail loads ride the scalar HWDGE queue so the sync queue
            # drains before the half-1 boundary prefill needs it
            NTAIL = 3
            tails = []
            for i in range(NT - NTAIL, NT):
                xt = stream2.tile([128, V], F32, tag="xt2")
                nc.scalar.dma_start(out=xt[:],
                                    in_=acts_rows[i * 128:(i + 1) * 128, :])
                tails.append(xt)
            for i in range(NT):
                if i < NT - NTAIL:
                    xt = stream.tile([128, V], F32, tag="xt")
                    nc.sync.dma_start(out=xt[:],
                                      in_=acts_rows[i * 128:(i + 1) * 128, :])
                else:
                    xt = tails[i - (NT - NTAIL)]
                ex = psump.tile([128, V], F32, tag="ex")
                nc.scalar.activation(
                    out=ex[:], in_=xt[:],
                    func=mybir.ActivationFunctionType.Exp,
                    bias=zbias[:], scale=1.0,
                    accum_out=sums[:, i:i + 1])

            # ---------------- wavefront halves ----------------
            for h, hs in enumerate(HALVES):
                TB, CWW, NSLOT, NB = hs["tb"], hs["cww"], hs["nslot"], hs["nb"]
                NROW = 4 * NB                # rows at partitions 0..NROW-1

                def GOFF(j):
                    return (PADS + j) * CWW

                def STO(j):
                    return (PADS + j) * CWW + 1

                Xh, Eh, SKh = X[h], E[h], SK[h]
                if h == 1:
                    # boundary prefill FIRST on the gpsimd queue as ONE
                    # contiguous block (the strided 4B-element form cost
                    # ~9us of tiny DMA packets); in-between cells of the
                    # constants partitions are dont-cares
                    nc.sync.dma_start(
                        out=Xh[28:32, TB:TB + (NSLOT + 2) * CWW],
                        in_=bscw[:])
                    nc.gpsimd.dma_start(out=rfac_d[:], in_=r_t[:])
                    nc.gpsimd.dma_start(
                        out=xsk_d[0][:],
                        in_=X[0][:, PADS * HALVES[0]["cww"]:])
                for sg in range(NSLOT):
                    # seam shuffle (all 32 partitions; wrap feeds row 0
                    # from the constants group)
                    nc.vector.stream_shuffle(
                        out=Xh[:, GOFF(sg - 2):GOFF(sg) + 1:CWW],
                        in_=Xh[:, GOFF(sg - 2) - 1:GOFF(sg):CWW],
                        mask=mask_dn4)
                    # drive u_tau = x^{s-1}_{t-1} + m_s * x^{s-2}_{t-1}
                    nc.vector.scalar_tensor_tensor(
                        out=u_t[0:NROW, 0:TB],
                        in0=Xh[0:NROW, GOFF(sg - 2):GOFF(sg - 2) + TB],
                        scalar=SKh[0:NROW, sg:sg + 1],
                        in1=Xh[0:NROW, GOFF(sg - 1):GOFF(sg - 1) + TB],
                        op0=MUL, op1=ADD)
                    # x_tau = (u_tau + x_{tau-1}) * E_tau
                    nc.vector.tensor_tensor_scan(
                        out=Xh[0:NROW, STO(sg):STO(sg) + TB],
                        data0=u_t[0:NROW, 0:TB],
                        data1=Eh[0:NROW, sg * TB:(sg + 1) * TB],
                        initial=Xh[0:NROW, GOFF(sg):GOFF(sg) + 1],
                        op0=ADD, op1=MUL)
                if h == 1:
                    nc.gpsimd.dma_start(out=xsk_d[1][:],
                                        in_=Xh[:, PADS * CWW:])
                    nc.gpsimd.dma_start(out=sums_d[:], in_=sums[:])
                if h == 0:
                    # boundary x^s_{223} on partitions 24..27 (row 6);
                    # shuffle up to 0..3, renorm max -> e^TB_LOG
                    nc.vector.stream_shuffle(
                        out=bndt[:],
                        in_=Xh[:, STO(NB - 1) + TB - 1:
                               STO(NB - 1 + S - 1) + TB:CWW],
                        mask=mask_up24)
                    nc.vector.reduce_max(out=m_t[:], in_=bndt[0:BC, :],
                                         axis=mybir.AxisListType.X)
                    nc.vector.reciprocal(out=r0_t[:], in_=m_t[:])
                    nc.vector.tensor_scalar_mul(r_t[:], r0_t[:],
                                                float(np.exp(TB_LOG)))
                    cw1 = HALVES[1]["cww"]
                    nc.vector.tensor_scalar_mul(
                        bscw[:, 2 * cw1:(2 + S) * cw1:cw1],
                        bndt[0:BC, :], r_t[:, 0:1])

    nc.compile()
    return nc


def _get_nc():
    if "nc" not in _CACHE:
        _CACHE["nc"] = _build_nc()
    return _CACHE["nc"]


def host_prep(acts, labels, act_lens, label_lens):
    """Build the 8 per-core input maps (skew-laid emissions + masks)."""
    acts = np.ascontiguousarray(np.asarray(acts, dtype=np.float32))
    labels = np.asarray(labels).astype(np.int64)
    al = np.asarray(act_lens).astype(np.int64)
    ll = np.asarray(label_lens).astype(np.int64)
    offsets = np.cumsum(ll) - ll
    in_maps = []
    for k in range(NCORES):
        bsl = slice(k * BC, (k + 1) * BC)
        slab = np.ascontiguousarray(acts[:, bsl, :])
        gmax = np.zeros((BC, T), np.float64)
        gt = np.zeros((BC, T, S), np.float32)
        skipm0 = np.zeros((BC, S), np.float32)
        for bl in range(BC):
            b = k * BC + bl
            L = int(ll[b])
            lab = np.zeros(LMAX, np.int64)
            lab[:L] = labels[offsets[b]: offsets[b] + L]
            ext = np.zeros(S, np.int64)
            ext[1::2] = lab
            g = slab[:, bl, ext].astype(np.float64)
            gm = g.max(axis=1)
            gmax[bl] = gm
            gt[bl] = (g - gm[:, None]).astype(np.float32)
            skipm0[bl, 1] = 1.0
            for jj in range(1, L):
                if lab[jj] != lab[jj - 1]:
                    skipm0[bl, 2 * jj + 1] = 1.0
        seed = np.zeros((BC, (HALVES[0]["nslot"] + 2)
                         * HALVES[0]["cww"]), np.float32)
        seed[:, HALVES[0]["cww"]] = 1.0     # gap G(-1): x^{-1}_{-1} = 1
        im = {"acts": slab, "seed": seed, "_gmax": gmax}
        for h, hs in enumerate(HALVES):
            NB, TB, NSLOT = hs["nb"], hs["tb"], hs["nslot"]
            gsk = np.full((8, BC, NSLOT, TB), -1e30, np.float32)
            skm = np.zeros((8, BC, NSLOT), np.float32)
            for kk in range(NB):
                blk = gt[:, hs["t0"] + kk * TB:hs["t0"] + (kk + 1) * TB, :]
                gsk[kk, :, kk:kk + S, :] = blk.transpose(0, 2, 1)
                skm[kk, :, kk:kk + S] = skipm0
            im[f"gsk{h}"] = gsk.reshape(32, NSLOT * TB)
            im[f"skm{h}"] = skm.reshape(32, NSLOT)
        in_maps.append(im)
    return in_maps, al, ll


def _readout(xsk, bl, s, t):
    """Value x^s_t from the dumped skewed layouts."""
    h = 0 if t < TH0 else 1
    hs = HALVES[h]
    tb = t - hs["t0"]
    kk = tb // hs["tb"]
    tau = tb % hs["tb"]
    return xsk[h][4 * kk + bl, (s + kk) * hs["cww"] + 1 + tau]


def host_finalize(results, al, ll, gmaxes):
    total = np.float64(0.0)
    for k in range(NCORES):
        r = results[k]
        sums = np.asarray(r["sums"], np.float64)
        xsk = [np.asarray(r["xsk0"], np.float64),
               np.asarray(r["xsk1"], np.float64)]
        rfac = np.asarray(r["rfac"], np.float64)
        gmax = gmaxes[k]
        lse_rows = np.log(sums.T.reshape(-1)).reshape(T, BC)
        for bl in range(BC):
            b = k * BC + bl
            L = int(ll[b])
            albb = int(al[b])
            t_star = albb - 1
            e_s = 2 * L
            rs = (_readout(xsk, bl, e_s, t_star)
                  + _readout(xsk, bl, e_s - 1, t_star))
            log_unnorm = (np.log(rs) + gmax[bl, :t_star + 1].sum()
                          + C_TILT * (t_star + 1))
            if t_star >= TH0:
                log_unnorm -= np.log(rfac[bl, 0])
            loss_b = -log_unnorm + lse_rows[:albb, bl].sum()
            total += loss_b
    return np.array([total], dtype=np.float32)


def kernel(acts, labels, act_lens, label_lens):
    from concourse.bass_utils import run_bass_kernel_spmd
    in_maps, al, ll = host_prep(acts, labels, act_lens, label_lens)
    gmaxes = [m.pop("_gmax") for m in in_maps]
    nc = _get_nc()
    res = run_bass_kernel_spmd(nc, in_maps, list(range(NCORES)))
    return host_finalize(res.results, al, ll, gmaxes)


# revision 18
# speedup vs baseline: 1.1056x; 1.1056x over previous
"""CTC total-loss kernel for Trainium2 (8 NeuronCores) — wavefront v2.

Wavefront alpha recursion with a dead "constants" partition group:

 * Halves t<224 (NB=7 rows x TB=32) and t>=224 (NB=6 rows x TB=48).
   Block-row k lives in partitions 4(k+1)..4(k+1)+3; partitions 0..3 are a
   never-scanned constants group.
 * One stream_shuffle per diagonal moves seam state down 4 partitions into
   gap cells; the wrap at partition group 0 reads the constants group, so
   row 0's virtual t=-1 seed (half 0) / renormed boundary (half 1) arrives
   with NO extra per-diagonal op (one strided prefill per half instead).
 * Per diagonal: shuffle + stt (drive) + scan.  141 diagonals total.
 * Renorm at t=224 (boundary max -> e^TB_LOG), compensated on host.
"""

import numpy as np

import concourse.bass as bass
import concourse.bacc as bacc
import concourse.tile as tile
from concourse import mybir

F32 = mybir.dt.float32
BF16 = mybir.dt.bfloat16

T, B, V, LMAX = 512, 32, 4096, 32
NCORES = 8
BC = B // NCORES            # 4 examples per core
S = 2 * LMAX + 1            # 65 lattice states
NT = (T * BC) // 128        # 16 stream tiles of (128, V)
C_TILT = -1.20
TB_LOG = 58.0

# two asymmetric halves; each: NB rows of TB, rows at partitions 4..4+4*NB-1
HALVES = (
    dict(t0=0, nb=7, tb=32),
    dict(t0=224, nb=6, tb=48),
)
for hs in HALVES:
    hs["th"] = hs["nb"] * hs["tb"]
    hs["cww"] = hs["tb"] + 1
    hs["nslot"] = S + hs["nb"] - 1
    hs["xw"] = (3 + hs["nslot"]) * hs["cww"]   # 3 pad slots
PADS = 3
TH0 = HALVES[0]["th"]       # renorm boundary time

_CACHE = {}


def _build_nc():
    nc = bacc.Bacc(None)
    acts_d = nc.dram_tensor("acts", [T, BC, V], F32, kind="ExternalInput")
    gsk_d = [nc.dram_tensor(f"gsk{h}", [32, hs["nslot"] * hs["tb"]], F32,
                            kind="ExternalInput")
             for h, hs in enumerate(HALVES)]
    skm_d = [nc.dram_tensor(f"skm{h}", [32, hs["nslot"]], F32,
                            kind="ExternalInput")
             for h, hs in enumerate(HALVES)]
    xsk_d = [nc.dram_tensor(f"xsk{h}", [32, hs["nslot"] * hs["cww"]], F32,
                            kind="ExternalOutput")
             for h, hs in enumerate(HALVES)]
    seed_d = nc.dram_tensor("seed", [BC, (HALVES[0]["nslot"] + 2)
                            * HALVES[0]["cww"]], F32, kind="ExternalInput")
    rfac_d = nc.dram_tensor("rfac", [BC, 1], F32, kind="ExternalOutput")
    sums_d = nc.dram_tensor("sums", [128, NT], F32, kind="ExternalOutput")

    acts_rows = acts_d[:].rearrange("t b v -> (t b) v")     # (2048, 4096)

    ADD = mybir.AluOpType.add
    MUL = mybir.AluOpType.mult
    mask_dn4 = [(i - 4) % 32 for i in range(32)]
    mask_up24 = [(i + 24) % 32 for i in range(32)]

    with tile.TileContext(nc) as tc:
        with (
            tc.tile_pool(name="small", bufs=1) as small,
            tc.tile_pool(name="big", bufs=1) as big,
            tc.tile_pool(name="stream", bufs=3) as stream,
            tc.tile_pool(name="psum", bufs=1, space="PSUM") as psump,
        ):
            # ---------------- persistent tiles ----------------
            E = [big.tile([32, hs["nslot"] * hs["tb"]], BF16, tag=f"E{h}",
                          name=f"E{h}") for h, hs in enumerate(HALVES)]
            G = [big.tile([32, hs["nslot"] * hs["tb"]], F32, tag=f"G{h}",
                          name=f"G{h}") for h, hs in enumerate(HALVES)]
            X = [big.tile([32, hs["xw"]], F32, tag=f"X{h}",
                          name=f"X{h}") for h, hs in enumerate(HALVES)]
            SK = [small.tile([32, hs["nslot"]], F32, tag=f"SK{h}",
                           name=f"SK{h}") for h, hs in enumerate(HALVES)]
            u_t = big.tile([32, max(hs["tb"] for hs in HALVES)], F32, tag="u")

            negc = small.tile([32, 1], F32, tag="negc")
            zbias = small.tile([128, 1], F32, tag="zbias")
            bscw = small.tile([BC, (HALVES[1]["nslot"] + 2)
                               * HALVES[1]["cww"]], F32, tag="bscw")
            bndt = small.tile([32, S], F32, tag="bndt")
            m_t = small.tile([BC, 1], F32, tag="m")
            r0_t = small.tile([BC, 1], F32, tag="r0")
            r_t = small.tile([BC, 1], F32, tag="r")
            sums = small.tile([128, NT], F32, tag="sums")

            warm = small.tile([BC, 4], F32, tag="warm")
            nc.gpsimd.dma_start(out=warm[:], in_=seed_d[:, 0:4])
            # X0 memset first: it gates the half-0 constants prefill
            nc.vector.memset(X[0][:, :], 0.0)
            nc.vector.memset(X[1][:, :], 0.0)
            nc.vector.memset(negc[:], -C_TILT)
            nc.vector.memset(zbias[:], 0.0)
            nc.vector.memset(bscw[:], 0.0)
            # prep loads + half-0 constants prefill ride the sync queue
            # ahead of the stream loads (seed is host-filled: no DVE dep
            # beyond the X0 memset)
            for h in range(2):
                nc.sync.dma_start(out=G[h][:], in_=gsk_d[h][:])
                nc.sync.dma_start(out=SK[h][:], in_=skm_d[h][:])
            nc.sync.dma_start(
                out=X[0][28:32,
                         HALVES[0]["tb"]:HALVES[0]["tb"]
                         + (HALVES[0]["nslot"] + 2) * HALVES[0]["cww"]],
                in_=seed_d[:])
            e0cut = 8 * HALVES[0]["tb"]
            nc.scalar.activation(out=E[0][:, 0:e0cut], in_=G[0][:, 0:e0cut],
                                 func=mybir.ActivationFunctionType.Exp,
                                 bias=negc[:], scale=1.0)
            nc.scalar.activation(out=E[0][:, e0cut:], in_=G[0][:, e0cut:],
                                 func=mybir.ActivationFunctionType.Exp,
                                 bias=negc[:], scale=1.0)
            nc.scalar.activation(out=E[1][:], in_=G[1][:],
                                 func=mybir.ActivationFunctionType.Exp,
                                 bias=negc[:], scale=1.0)

            # ---------------- lse stream (DMA-roofline bound) ----------
            for i in range(NT):
                xt = stream.tile([128, V], F32, tag="xt")
                nc.sync.dma_start(out=xt[:],
                                  in_=acts_rows[i * 128:(i + 1) * 128, :])
                ex = psump.tile([128, V], F32, tag="ex")
                nc.scalar.activation(
                    out=ex[:], in_=xt[:],
                    func=mybir.ActivationFunctionType.Exp,
                    bias=zbias[:], scale=1.0,
                    accum_out=sums[:, i:i + 1])

            # ---------------- wavefront halves ----------------
            for h, hs in enumerate(HALVES):
                TB, CWW, NSLOT, NB = hs["tb"], hs["cww"], hs["nslot"], hs["nb"]
                NROW = 4 * NB                # rows at partitions 0..NROW-1

                def GOFF(j):
                    return (PADS + j) * CWW

                def STO(j):
                    return (PADS + j) * CWW + 1

                Xh, Eh, SKh = X[h], E[h], SK[h]
                if h == 1:
                    # boundary prefill on gpsimd in two contiguous pieces:
                    # a small head (constants for diagonals 0..7) unblocks
                    # the chain early; the bulk lands behind those diags
                    HEADC = 10
                    nc.gpsimd.dma_start(
                        out=Xh[28:32, TB:TB + HEADC * CWW],
                        in_=bscw[:, 0:HEADC * CWW])
                    nc.gpsimd.dma_start(
                        out=Xh[28:32, TB + HEADC * CWW:
                               TB + (NSLOT + 2) * CWW],
                        in_=bscw[:, HEADC * CWW:])
                    nc.gpsimd.dma_start(out=rfac_d[:], in_=r_t[:])
                    nc.gpsimd.dma_start(
                        out=xsk_d[0][:],
                        in_=X[0][:, PADS * HALVES[0]["cww"]:])
                for sg in range(NSLOT):
                    if h == 1 and sg == 45:
                        # slots 0..39 are final: dump them mid-chain so
                        # only ~half the dump sits in the program tail
                        nc.gpsimd.dma_start(
                            out=xsk_d[1][:, 0:40 * CWW],
                            in_=Xh[:, PADS * CWW:(PADS + 40) * CWW])
                    # seam shuffle (all 32 partitions; wrap feeds row 0
                    # from the constants group)
                    nc.vector.stream_shuffle(
                        out=Xh[:, GOFF(sg - 2):GOFF(sg) + 1:CWW],
                        in_=Xh[:, GOFF(sg - 2) - 1:GOFF(sg):CWW],
                        mask=mask_dn4)
                    # drive u_tau = x^{s-1}_{t-1} + m_s * x^{s-2}_{t-1}
                    nc.vector.scalar_tensor_tensor(
                        out=u_t[0:NROW, 0:TB],
                        in0=Xh[0:NROW, GOFF(sg - 2):GOFF(sg - 2) + TB],
                        scalar=SKh[0:NROW, sg:sg + 1],
                        in1=Xh[0:NROW, GOFF(sg - 1):GOFF(sg - 1) + TB],
                        op0=MUL, op1=ADD)
                    # x_tau = (u_tau + x_{tau-1}) * E_tau
                    nc.vector.tensor_tensor_scan(
                        out=Xh[0:NROW, STO(sg):STO(sg) + TB],
                        data0=u_t[0:NROW, 0:TB],
                        data1=Eh[0:NROW, sg * TB:(sg + 1) * TB],
                        initial=Xh[0:NROW, GOFF(sg):GOFF(sg) + 1],
                        op0=ADD, op1=MUL)
                if h == 1:
                    nc.gpsimd.dma_start(
                        out=xsk_d[1][:, 40 * CWW:],
                        in_=Xh[:, (PADS + 40) * CWW:])
                    nc.gpsimd.dma_start(out=sums_d[:], in_=sums[:])
                if h == 0:
                    # boundary x^s_{223} on partitions 24..27 (row 6);
                    # shuffle up to 0..3, renorm max -> e^TB_LOG
                    nc.vector.stream_shuffle(
                        out=bndt[:],
                        in_=Xh[:, STO(NB - 1) + TB - 1:
                               STO(NB - 1 + S - 1) + TB:CWW],
                        mask=mask_up24)
                    nc.vector.reduce_max(out=m_t[:], in_=bndt[0:BC, :],
                                         axis=mybir.AxisListType.X)
                    nc.vector.reciprocal(out=r0_t[:], in_=m_t[:])
                    nc.vector.tensor_scalar_mul(r_t[:], r0_t[:],
                                                float(np.exp(TB_LOG)))
                    cw1 = HALVES[1]["cww"]
                    nc.vector.tensor_scalar_mul(
                        bscw[:, 2 * cw1:(2 + S) * cw1:cw1],
                        bndt[0:BC, :], r_t[:, 0:1])

    nc.compile()
    return nc


def _get_nc():
    if "nc" not in _CACHE:
        _CACHE["nc"] = _build_nc()
    return _CACHE["nc"]


def host_prep(acts, labels, act_lens, label_lens):
    """Build the 8 per-core input maps (skew-laid emissions + masks)."""
    acts = np.ascontiguousarray(np.asarray(acts, dtype=np.float32))
    labels = np.asarray(labels).astype(np.int64)
    al = np.asarray(act_lens).astype(np.int64)
    ll = np.asarray(label_lens).astype(np.int64)
    offsets = np.cumsum(ll) - ll
    in_maps = []
    for k in range(NCORES):
        bsl = slice(k * BC, (k + 1) * BC)
        slab = np.ascontiguousarray(acts[:, bsl, :])
        gmax = np.zeros((BC, T), np.float64)
        gt = np.zeros((BC, T, S), np.float32)
        skipm0 = np.zeros((BC, S), np.float32)
        for bl in range(BC):
            b = k * BC + bl
            L = int(ll[b])
            lab = np.zeros(LMAX, np.int64)
            lab[:L] = labels[offsets[b]: offsets[b] + L]
            ext = np.zeros(S, np.int64)
            ext[1::2] = lab
            g = slab[:, bl, ext].astype(np.float64)
            gm = g.max(axis=1)
            gmax[bl] = gm
            gt[bl] = (g - gm[:, None]).astype(np.float32)
            skipm0[bl, 1] = 1.0
            for jj in range(1, L):
                if lab[jj] != lab[jj - 1]:
                    skipm0[bl, 2 * jj + 1] = 1.0
        seed = np.zeros((BC, (HALVES[0]["nslot"] + 2)
                         * HALVES[0]["cww"]), np.float32)
        seed[:, HALVES[0]["cww"]] = 1.0     # gap G(-1): x^{-1}_{-1} = 1
        im = {"acts": slab, "seed": seed, "_gmax": gmax}
        for h, hs in enumerate(HALVES):
            NB, TB, NSLOT = hs["nb"], hs["tb"], hs["nslot"]
            gsk = np.full((8, BC, NSLOT, TB), -1e30, np.float32)
            skm = np.zeros((8, BC, NSLOT), np.float32)
            for kk in range(NB):
                blk = gt[:, hs["t0"] + kk * TB:hs["t0"] + (kk + 1) * TB, :]
                gsk[kk, :, kk:kk + S, :] = blk.transpose(0, 2, 1)
                skm[kk, :, kk:kk + S] = skipm0
            im[f"gsk{h}"] = gsk.reshape(32, NSLOT * TB)
            im[f"skm{h}"] = skm.reshape(32, NSLOT)
        in_maps.append(im)
    return in_maps, al, ll


def _readout(xsk, bl, s, t):
    """Value x^s_t from the dumped skewed layouts."""
    h = 0 if t < TH0 else 1
    hs = HALVES[h]
    tb = t - hs["t0"]
    kk = tb // hs["tb"]
    tau = tb % hs["tb"]
    return xsk[h][4 * kk + bl, (s + kk) * hs["cww"] + 1 + tau]


def host_finalize(results, al, ll, gmaxes):
    total = np.float64(0.0)
    for k in range(NCORES):
        r = results[k]
        sums = np.asarray(r["sums"], np.float64)
        xsk = [np.asarray(r["xsk0"], np.float64),
               np.asarray(r["xsk1"], np.float64)]
        rfac = np.asarray(r["rfac"], np.float64)
        gmax = gmaxes[k]
        lse_rows = np.log(sums.T.reshape(-1)).reshape(T, BC)
        for bl in range(BC):
            b = k * BC + bl
            L = int(ll[b])
            albb = int(al[b])
            t_star = albb - 1
            e_s = 2 * L
            rs = (_readout(xsk, bl, e_s, t_star)
                  + _readout(xsk, bl, e_s - 1, t_star))
            log_unnorm = (np.log(rs) + gmax[bl, :t_star + 1].sum()
                          + C_TILT * (t_star + 1))
            if t_star >= TH0:
                log_unnorm -= np.log(rfac[bl, 0])
            loss_b = -log_unnorm + lse_rows[:albb, bl].sum()
            total += loss_b
    return np.array([total], dtype=np.float32)


def kernel(acts, labels, act_lens, label_lens):
    from concourse.bass_utils import run_bass_kernel_spmd
    in_maps, al, ll = host_prep(acts, labels, act_lens, label_lens)
    gmaxes = [m.pop("_gmax") for m in in_maps]
    nc = _get_nc()
    res = run_bass_kernel_spmd(nc, in_maps, list(range(NCORES)))
    return host_finalize(res.results, al, ll, gmaxes)


# revision 19
# speedup vs baseline: 1.1162x; 1.0096x over previous
"""CTC total-loss kernel for Trainium2 (8 NeuronCores) — wavefront v2.

Wavefront alpha recursion with a dead "constants" partition group:

 * Halves t<224 (NB=7 rows x TB=32) and t>=224 (NB=6 rows x TB=48).
   Block-row k lives in partitions 4(k+1)..4(k+1)+3; partitions 0..3 are a
   never-scanned constants group.
 * One stream_shuffle per diagonal moves seam state down 4 partitions into
   gap cells; the wrap at partition group 0 reads the constants group, so
   row 0's virtual t=-1 seed (half 0) / renormed boundary (half 1) arrives
   with NO extra per-diagonal op (one strided prefill per half instead).
 * Per diagonal: shuffle + stt (drive) + scan.  141 diagonals total.
 * Renorm at t=224 (boundary max -> e^TB_LOG), compensated on host.
"""

import numpy as np

import concourse.bass as bass
import concourse.bacc as bacc
import concourse.tile as tile
from concourse import mybir

F32 = mybir.dt.float32
BF16 = mybir.dt.bfloat16

T, B, V, LMAX = 512, 32, 4096, 32
NCORES = 8
BC = B // NCORES            # 4 examples per core
S = 2 * LMAX + 1            # 65 lattice states
NT = (T * BC) // 128        # 16 stream tiles of (128, V)
C_TILT = -1.20
TB_LOG = 58.0

# two asymmetric halves; each: NB rows of TB, rows at partitions 4..4+4*NB-1
HALVES = (
    dict(t0=0, nb=7, tb=32),
    dict(t0=224, nb=6, tb=48),
)
for hs in HALVES:
    hs["th"] = hs["nb"] * hs["tb"]
    hs["cww"] = hs["tb"] + 1
    hs["nslot"] = S + hs["nb"] - 1
    hs["xw"] = (3 + hs["nslot"]) * hs["cww"]   # 3 pad slots
PADS = 3
TH0 = HALVES[0]["th"]       # renorm boundary time

_CACHE = {}


def _build_nc():
    nc = bacc.Bacc(None)
    acts_d = nc.dram_tensor("acts", [T, BC, V], F32, kind="ExternalInput")
    gsk_d = [nc.dram_tensor(f"gsk{h}", [32, hs["nslot"] * hs["tb"]], F32,
                            kind="ExternalInput")
             for h, hs in enumerate(HALVES)]
    skm_d = [nc.dram_tensor(f"skm{h}", [32, hs["nslot"]], F32,
                            kind="ExternalInput")
             for h, hs in enumerate(HALVES)]
    xsk_d = [nc.dram_tensor(f"xsk{h}", [32, hs["nslot"] * hs["cww"]], F32,
                            kind="ExternalOutput")
             for h, hs in enumerate(HALVES)]
    seed_d = nc.dram_tensor("seed", [BC, (HALVES[0]["nslot"] + 2)
                            * HALVES[0]["cww"]], F32, kind="ExternalInput")
    rfac_d = nc.dram_tensor("rfac", [BC, 1], F32, kind="ExternalOutput")
    sums_d = nc.dram_tensor("sums", [128, NT], F32, kind="ExternalOutput")

    acts_rows = acts_d[:].rearrange("t b v -> (t b) v")     # (2048, 4096)

    ADD = mybir.AluOpType.add
    MUL = mybir.AluOpType.mult
    mask_dn4 = [(i - 4) % 32 for i in range(32)]
    mask_up24 = [(i + 24) % 32 for i in range(32)]

    with tile.TileContext(nc) as tc:
        with (
            tc.tile_pool(name="small", bufs=1) as small,
            tc.tile_pool(name="big", bufs=1) as big,
            tc.tile_pool(name="stream", bufs=3) as stream,
            tc.tile_pool(name="psum", bufs=1, space="PSUM") as psump,
        ):
            # ---------------- persistent tiles ----------------
            E = [big.tile([32, hs["nslot"] * hs["tb"]], BF16, tag=f"E{h}",
                          name=f"E{h}") for h, hs in enumerate(HALVES)]
            G = [big.tile([32, hs["nslot"] * hs["tb"]], F32, tag=f"G{h}",
                          name=f"G{h}") for h, hs in enumerate(HALVES)]
            X = [big.tile([32, hs["xw"]], F32, tag=f"X{h}",
                          name=f"X{h}") for h, hs in enumerate(HALVES)]
            SK = [small.tile([32, hs["nslot"]], F32, tag=f"SK{h}",
                           name=f"SK{h}") for h, hs in enumerate(HALVES)]
            u_t = big.tile([32, max(hs["tb"] for hs in HALVES)], F32, tag="u")

            negc = small.tile([32, 1], F32, tag="negc")
            zbias = small.tile([128, 1], F32, tag="zbias")
            bscw = small.tile([BC, (HALVES[1]["nslot"] + 2)
                               * HALVES[1]["cww"]], F32, tag="bscw")
            bndt = small.tile([32, S], F32, tag="bndt")
            m_t = small.tile([BC, 1], F32, tag="m")
            r0_t = small.tile([BC, 1], F32, tag="r0")
            r_t = small.tile([BC, 1], F32, tag="r")
            sums = small.tile([128, NT], F32, tag="sums")

            warm = small.tile([BC, 4], F32, tag="warm")
            nc.gpsimd.dma_start(out=warm[:], in_=seed_d[:, 0:4])
            # X0 memset first: it gates the half-0 constants prefill
            nc.vector.memset(X[0][:, :], 0.0)
            nc.vector.memset(X[1][:, :], 0.0)
            nc.vector.memset(negc[:], -C_TILT)
            nc.vector.memset(zbias[:], 0.0)
            nc.vector.memset(bscw[:], 0.0)
            # prep loads + half-0 constants prefill ride the sync queue
            # ahead of the stream loads (seed is host-filled: no DVE dep
            # beyond the X0 memset)
            for h in range(2):
                nc.sync.dma_start(out=G[h][:], in_=gsk_d[h][:])
                nc.sync.dma_start(out=SK[h][:], in_=skm_d[h][:])
            nc.sync.dma_start(
                out=X[0][28:32,
                         HALVES[0]["tb"]:HALVES[0]["tb"]
                         + (HALVES[0]["nslot"] + 2) * HALVES[0]["cww"]],
                in_=seed_d[:])
            e0cut = 8 * HALVES[0]["tb"]
            nc.scalar.activation(out=E[0][:, 0:e0cut], in_=G[0][:, 0:e0cut],
                                 func=mybir.ActivationFunctionType.Exp,
                                 bias=negc[:], scale=1.0)
            nc.scalar.activation(out=E[0][:, e0cut:], in_=G[0][:, e0cut:],
                                 func=mybir.ActivationFunctionType.Exp,
                                 bias=negc[:], scale=1.0)
            nc.scalar.activation(out=E[1][:], in_=G[1][:],
                                 func=mybir.ActivationFunctionType.Exp,
                                 bias=negc[:], scale=1.0)

            # ---------------- lse stream (DMA-roofline bound) ----------
            for i in range(NT):
                xt = stream.tile([128, V], F32, tag="xt")
                nc.sync.dma_start(out=xt[:],
                                  in_=acts_rows[i * 128:(i + 1) * 128, :])
                ex = psump.tile([128, V], F32, tag="ex")
                nc.scalar.activation(
                    out=ex[:], in_=xt[:],
                    func=mybir.ActivationFunctionType.Exp,
                    bias=zbias[:], scale=1.0,
                    accum_out=sums[:, i:i + 1])

            # ---------------- wavefront halves ----------------
            for h, hs in enumerate(HALVES):
                TB, CWW, NSLOT, NB = hs["tb"], hs["cww"], hs["nslot"], hs["nb"]
                NROW = 4 * NB                # rows at partitions 0..NROW-1

                def GOFF(j):
                    return (PADS + j) * CWW

                def STO(j):
                    return (PADS + j) * CWW + 1

                Xh, Eh, SKh = X[h], E[h], SK[h]
                if h == 1:
                    # boundary prefill FIRST on the gpsimd queue as ONE
                    # contiguous block (the strided 4B-element form cost
                    # ~9us of tiny DMA packets); in-between cells of the
                    # constants partitions are dont-cares
                    nc.gpsimd.dma_start(
                        out=Xh[28:32, TB:TB + (NSLOT + 2) * CWW],
                        in_=bscw[:])
                    nc.gpsimd.dma_start(out=rfac_d[:], in_=r_t[:])
                    nc.gpsimd.dma_start(
                        out=xsk_d[0][:],
                        in_=X[0][:, PADS * HALVES[0]["cww"]:])
                for sg in range(NSLOT):
                    # seam shuffle (all 32 partitions; wrap feeds row 0
                    # from the constants group)
                    nc.vector.stream_shuffle(
                        out=Xh[:, GOFF(sg - 2):GOFF(sg) + 1:CWW],
                        in_=Xh[:, GOFF(sg - 2) - 1:GOFF(sg):CWW],
                        mask=mask_dn4)
                    # drive u_tau = x^{s-1}_{t-1} + m_s * x^{s-2}_{t-1}
                    nc.vector.scalar_tensor_tensor(
                        out=u_t[0:NROW, 0:TB],
                        in0=Xh[0:NROW, GOFF(sg - 2):GOFF(sg - 2) + TB],
                        scalar=SKh[0:NROW, sg:sg + 1],
                        in1=Xh[0:NROW, GOFF(sg - 1):GOFF(sg - 1) + TB],
                        op0=MUL, op1=ADD)
                    # x_tau = (u_tau + x_{tau-1}) * E_tau
                    nc.vector.tensor_tensor_scan(
                        out=Xh[0:NROW, STO(sg):STO(sg) + TB],
                        data0=u_t[0:NROW, 0:TB],
                        data1=Eh[0:NROW, sg * TB:(sg + 1) * TB],
                        initial=Xh[0:NROW, GOFF(sg):GOFF(sg) + 1],
                        op0=ADD, op1=MUL)
                if h == 1:
                    nc.gpsimd.dma_start(out=xsk_d[1][:],
                                        in_=Xh[:, PADS * CWW:])
                    nc.gpsimd.dma_start(out=sums_d[:], in_=sums[:])
                if h == 0:
                    # boundary x^s_{223} on partitions 24..27 (row 6);
                    # shuffle up to 0..3, renorm max -> e^TB_LOG
                    nc.vector.stream_shuffle(
                        out=bndt[:],
                        in_=Xh[:, STO(NB - 1) + TB - 1:
                               STO(NB - 1 + S - 1) + TB:CWW],
                        mask=mask_up24)
                    nc.vector.reduce_max(out=m_t[:], in_=bndt[0:BC, :],
                                         axis=mybir.AxisListType.X)
                    nc.vector.reciprocal(out=r0_t[:], in_=m_t[:])
                    nc.vector.tensor_scalar_mul(r_t[:], r0_t[:],
                                                float(np.exp(TB_LOG)))
                    cw1 = HALVES[1]["cww"]
                    nc.vector.tensor_scalar_mul(
                        bscw[:, 2 * cw1:(2 + S) * cw1:cw1],
                        bndt[0:BC, :], r_t[:, 0:1])

    nc.compile()
    return nc


def _get_nc():
    if "nc" not in _CACHE:
        _CACHE["nc"] = _build_nc()
    return _CACHE["nc"]


def host_prep(acts, labels, act_lens, label_lens):
    """Build the 8 per-core input maps (skew-laid emissions + masks)."""
    acts = np.ascontiguousarray(np.asarray(acts, dtype=np.float32))
    labels = np.asarray(labels).astype(np.int64)
    al = np.asarray(act_lens).astype(np.int64)
    ll = np.asarray(label_lens).astype(np.int64)
    offsets = np.cumsum(ll) - ll
    in_maps = []
    for k in range(NCORES):
        bsl = slice(k * BC, (k + 1) * BC)
        slab = np.ascontiguousarray(acts[:, bsl, :])
        gmax = np.zeros((BC, T), np.float64)
        gt = np.zeros((BC, T, S), np.float32)
        skipm0 = np.zeros((BC, S), np.float32)
        for bl in range(BC):
            b = k * BC + bl
            L = int(ll[b])
            lab = np.zeros(LMAX, np.int64)
            lab[:L] = labels[offsets[b]: offsets[b] + L]
            ext = np.zeros(S, np.int64)
            ext[1::2] = lab
            g = slab[:, bl, ext].astype(np.float64)
            gm = g.max(axis=1)
            gmax[bl] = gm
            gt[bl] = (g - gm[:, None]).astype(np.float32)
            skipm0[bl, 1] = 1.0
            for jj in range(1, L):
                if lab[jj] != lab[jj - 1]:
                    skipm0[bl, 2 * jj + 1] = 1.0
        seed = np.zeros((BC, (HALVES[0]["nslot"] + 2)
                         * HALVES[0]["cww"]), np.float32)
        seed[:, HALVES[0]["cww"]] = 1.0     # gap G(-1): x^{-1}_{-1} = 1
        im = {"acts": slab, "seed": seed, "_gmax": gmax}
        for h, hs in enumerate(HALVES):
            NB, TB, NSLOT = hs["nb"], hs["tb"], hs["nslot"]
            gsk = np.full((8, BC, NSLOT, TB), -1e30, np.float32)
            skm = np.zeros((8, BC, NSLOT), np.float32)
            for kk in range(NB):
                blk = gt[:, hs["t0"] + kk * TB:hs["t0"] + (kk + 1) * TB, :]
                gsk[kk, :, kk:kk + S, :] = blk.transpose(0, 2, 1)
                skm[kk, :, kk:kk + S] = skipm0
            im[f"gsk{h}"] = gsk.reshape(32, NSLOT * TB)
            im[f"skm{h}"] = skm.reshape(32, NSLOT)
        in_maps.append(im)
    return in_maps, al, ll


def _readout(xsk, bl, s, t):
    """Value x^s_t from the dumped skewed layouts."""
    h = 0 if t < TH0 else 1
    hs = HALVES[h]
    tb = t - hs["t0"]
    kk = tb // hs["tb"]
    tau = tb % hs["tb"]
    return xsk[h][4 * kk + bl, (s + kk) * hs["cww"] + 1 + tau]


def host_finalize(results, al, ll, gmaxes):
    total = np.float64(0.0)
    for k in range(NCORES):
        r = results[k]
        sums = np.asarray(r["sums"], np.float64)
        xsk = [np.asarray(r["xsk0"], np.float64),
               np.asarray(r["xsk1"], np.float64)]
        rfac = np.asarray(r["rfac"], np.float64)
        gmax = gmaxes[k]
        lse_rows = np.log(sums.T.reshape(-1)).reshape(T, BC)
        for bl in range(BC):
            b = k * BC + bl
            L = int(ll[b])
            albb = int(al[b])
            t_star = albb - 1
            e_s = 2 * L
            rs = (_readout(xsk, bl, e_s, t_star)
                  + _readout(xsk, bl, e_s - 1, t_star))
            log_unnorm = (np.log(rs) + gmax[bl, :t_star + 1].sum()
                          + C_TILT * (t_star + 1))
            if t_star >= TH0:
                log_unnorm -= np.log(rfac[bl, 0])
            loss_b = -log_unnorm + lse_rows[:albb, bl].sum()
            total += loss_b
    return np.array([total], dtype=np.float32)


def kernel(acts, labels, act_lens, label_lens):
    from concourse.bass_utils import run_bass_kernel_spmd
    in_maps, al, ll = host_prep(acts, labels, act_lens, label_lens)
    gmaxes = [m.pop("_gmax") for m in in_maps]
    nc = _get_nc()
    res = run_bass_kernel_spmd(nc, in_maps, list(range(NCORES)))
    return host_finalize(res.results, al, ll, gmaxes)
